# revision 1
# baseline (speedup 1.0000x reference)
"""Pairwise IoU (8192x8192) on 8 Trainium2 NeuronCores via Bass/Tile.

Sharding: boxes1 rows split across 8 cores (1024 rows each); boxes2
replicated. Output row-sharded, gathered on host.

Per-core device kernel, per [128, MT] output tile (5 DVE passes):
  rx    = relu(min(x2_i, X2_j) - max(x1_i, X1_j))   fused custom-DVE op
  ry    = same for y                                fused custom-DVE op
  inter = rx*ry                                     DVE tensor_tensor
  u     = a1_i + (a2_j+eps) - inter                 TENSOR ENGINE -> PSUM
          (rank-1 matmul + (-I) @ inter accumulate; no DVE cost)
  rinv  = reciprocal_approx_fast(u_psum)            custom-DVE op (~51 ULP)
  out   = inter * rinv                              DVE tensor_tensor
"""

import numpy as np

N = 8192
M = 8192
NCORES = 8
ROWS = N // NCORES  # rows of boxes1 per core
P = 128  # partitions
ITILES = ROWS // P  # 8 i-tiles per core
MT = 2048  # j-chunk width
JCHUNKS = M // MT
PS = 512  # psum bank width (fp32)
PCHUNKS = MT // PS
EPS = 1e-7

_COMPILED = {}


def _ensure_iou_edge():
    """Register the IOU_EDGE custom DVE op (idempotent)."""
    import concourse.dve_ops as dve_ops

    for op in dve_ops.OPS:
        if op.name == "IOU_EDGE":
            return op

    from concourse.dve_spec import Spec, Src0, Src1, C0, C1, relu, minn, maxx

    spec = Spec(
        body=relu(minn(Src1, C1) - maxx(Src0, C0)),
        reference=lambda in0, in1, s0, s1, imm2: np.maximum(
            np.minimum(in1, s1) - np.maximum(in0, s0), 0.0
        ).astype(np.float32),
    )
    op = dve_ops.DveOp(
        "IOU_EDGE",
        spec,
        subdim=False,
        uops_sha={"v3": "6891eb10878e1367", "v4": "ef621f43a8326356"},
    )
    dve_ops.OPS.append(op)
    dve_ops.CUSTOM_DVE_SPECS[op.name] = op.spec
    dve_ops._SUB_OPCODE_FOR_NAME[op.name] = (
        dve_ops._CUSTOM_DVE_ROW_BASE + len(dve_ops.OPS) - 1
    )
    return op


def _build_program():
    from contextlib import ExitStack

    import concourse.bacc as bacc
    import concourse.mybir as mybir
    import concourse.tile as tile
    from concourse.dve_ops import RECIPROCAL_APPROX_FAST, RECIP_APPROX_FAST_CONSTS

    iou_edge = _ensure_iou_edge()
    rc = RECIP_APPROX_FAST_CONSTS

    f32 = mybir.dt.float32
    nc = bacc.Bacc(
        "TRN2",
        target_bir_lowering=False,
        debug=False,
        enable_asserts=False,
        num_devices=NCORES,
    )

    # DRAM I/O. Broadcast tensors are host-replicated across partitions.
    x1b = nc.dram_tensor("x1b", [P, M], f32, kind="ExternalInput").ap()
    x2b = nc.dram_tensor("x2b", [P, M], f32, kind="ExternalInput").ap()
    y1b = nc.dram_tensor("y1b", [P, M], f32, kind="ExternalInput").ap()
    y2b = nc.dram_tensor("y2b", [P, M], f32, kind="ExternalInput").ap()
    # moving operand for the union matmul: row0 = ones, row1 = area2+eps
    a2e2 = nc.dram_tensor("a2e2", [2, M], f32, kind="ExternalInput").ap()
    # stationary for the union matmul: row0 = area1 (per row), row1 = ones
    sta = nc.dram_tensor("sta", [2, ROWS], f32, kind="ExternalInput").ap()
    # negated identity for the -inter accumulate
    negi = nc.dram_tensor("negi", [P, P], f32, kind="ExternalInput").ap()
    # Per-partition scalars: for i-tile t, columns t*5+k hold
    # (x1, x2, y1, y2, area1) of boxes1 row t*128+p.
    sc = nc.dram_tensor("sc", [P, ITILES * 5], f32, kind="ExternalInput").ap()
    out = nc.dram_tensor("out", [ROWS, M], f32, kind="ExternalOutput").ap()

    with tile.TileContext(nc) as tc, ExitStack() as ctx:
        bcast = ctx.enter_context(tc.tile_pool(name="bcast", bufs=2))
        scp = ctx.enter_context(tc.tile_pool(name="scp", bufs=1))
        work = ctx.enter_context(tc.tile_pool(name="work", bufs=2))
        interp = ctx.enter_context(tc.tile_pool(name="interp", bufs=3))
        outp = ctx.enter_context(tc.tile_pool(name="outp", bufs=3))
        psum = ctx.enter_context(
            tc.tile_pool(name="psum", bufs=2, space="PSUM")
        )

        sct = scp.tile([P, ITILES * 5], f32)
        nc.sync.dma_start(sct[:], sc[:])
        negit = scp.tile([P, P], f32)
        nc.sync.dma_start(negit[:], negi[:])
        stat = scp.tile([2, ROWS], f32)
        nc.sync.dma_start(stat[:], sta[:])
        a2e2t = scp.tile([2, M], f32)
        nc.sync.dma_start(a2e2t[:], a2e2[:])

        for j in range(JCHUNKS):
            j0 = j * MT
            x1c = bcast.tile([P, MT], f32, tag="x1c")
            x2c = bcast.tile([P, MT], f32, tag="x2c")
            y1c = bcast.tile([P, MT], f32, tag="y1c")
            y2c = bcast.tile([P, MT], f32, tag="y2c")
            nc.sync.dma_start(x1c[:], x1b[:, j0 : j0 + MT])
            nc.sync.dma_start(x2c[:], x2b[:, j0 : j0 + MT])
            nc.sync.dma_start(y1c[:], y1b[:, j0 : j0 + MT])
            nc.sync.dma_start(y2c[:], y2b[:, j0 : j0 + MT])

            for t in range(ITILES):
                c = t * 5
                s_x1 = sct[:, c : c + 1]
                s_x2 = sct[:, c + 1 : c + 2]
                s_y1 = sct[:, c + 2 : c + 3]
                s_y2 = sct[:, c + 3 : c + 4]
                s_a1 = sct[:, c + 4 : c + 5]

                rx = work.tile([P, MT], f32, tag="rx")
                ry = work.tile([P, MT], f32, tag="ry")
                inter = interp.tile([P, MT], f32, tag="inter")
                rinv = work.tile([P, MT], f32, tag="rinv")
                ot = outp.tile([P, MT], f32, tag="ot")

                nc.vector._custom_dve(
                    iou_edge, out=rx[:], in0=x1c[:], in1=x2c[:], s0=s_x1, s1=s_x2
                )
                nc.vector._custom_dve(
                    iou_edge, out=ry[:], in0=y1c[:], in1=y2c[:], s0=s_y1, s1=s_y2
                )
                nc.vector.tensor_mul(inter[:], rx[:], ry[:])

                # u = a1_i + (a2_j + eps) - inter on the tensor engine.
                pt = psum.tile([P, MT], f32, tag="pt")
                for pc in range(PCHUNKS):
                    c0 = pc * PS
                    nc.tensor.matmul(
                        pt[:, c0 : c0 + PS],
                        stat[:, t * P : (t + 1) * P],
                        a2e2t[:, j0 + c0 : j0 + c0 + PS],
                        start=True,
                        stop=False,
                    )
                for pc in range(PCHUNKS):
                    c0 = pc * PS
                    nc.tensor.matmul(
                        pt[:, c0 : c0 + PS],
                        negit[:],
                        inter[:, c0 : c0 + PS],
                        start=False,
                        stop=True,
                    )
                for pc in range(PCHUNKS):
                    c0 = pc * PS
                    nc.vector._custom_dve(
                        RECIPROCAL_APPROX_FAST,
                        out=rinv[:, c0 : c0 + PS],
                        in0=pt[:, c0 : c0 + PS],
                        s0=rc["s0"],
                        s1=rc["s1"],
                        imm2=rc["imm2"],
                    )

                nc.vector.tensor_mul(ot[:], inter[:], rinv[:])
                nc.sync.dma_start(out[t * P : (t + 1) * P, j0 : j0 + MT], ot[:])

    nc.compile()
    return nc


def _get_program():
    if "nc" not in _COMPILED:
        _COMPILED["nc"] = _build_program()
    return _COMPILED["nc"]


def _make_in_maps(boxes1, boxes2):
    boxes1 = np.ascontiguousarray(boxes1, dtype=np.float32)
    boxes2 = np.ascontiguousarray(boxes2, dtype=np.float32)

    a2e = (boxes2[:, 2] - boxes2[:, 0]) * (boxes2[:, 3] - boxes2[:, 1]) + np.float32(
        EPS
    )
    reps = {}
    for name, vec in (
        ("x1b", boxes2[:, 0]),
        ("x2b", boxes2[:, 2]),
        ("y1b", boxes2[:, 1]),
        ("y2b", boxes2[:, 3]),
    ):
        reps[name] = np.ascontiguousarray(
            np.broadcast_to(vec.astype(np.float32), (P, M))
        )
    a2e2 = np.stack([np.ones(M, np.float32), a2e]).astype(np.float32)
    negi = (-np.eye(P)).astype(np.float32)

    in_maps = []
    for c in range(NCORES):
        b1c = boxes1[c * ROWS : (c + 1) * ROWS].reshape(ITILES, P, 4)
        a1 = (b1c[:, :, 2] - b1c[:, :, 0]) * (b1c[:, :, 3] - b1c[:, :, 1])
        sta = np.stack([a1.reshape(ROWS), np.ones(ROWS, np.float32)]).astype(
            np.float32
        )
        sc = np.empty((P, ITILES * 5), dtype=np.float32)
        for t in range(ITILES):
            sc[:, t * 5 + 0] = b1c[t, :, 0]
            sc[:, t * 5 + 1] = b1c[t, :, 2]
            sc[:, t * 5 + 2] = b1c[t, :, 1]
            sc[:, t * 5 + 3] = b1c[t, :, 3]
            sc[:, t * 5 + 4] = a1[t]
        in_maps.append(
            {**reps, "a2e2": a2e2, "sta": sta, "negi": negi, "sc": sc}
        )
    return in_maps


def kernel(boxes1: np.ndarray, boxes2: np.ndarray) -> np.ndarray:
    from concourse.bass_utils import run_bass_kernel_spmd

    nc = _get_program()
    in_maps = _make_in_maps(boxes1, boxes2)
    res = run_bass_kernel_spmd(nc, in_maps, core_ids=list(range(NCORES)))
    return np.concatenate([res.results[c]["out"] for c in range(NCORES)], axis=0)



# revision 2
# speedup vs baseline: 3.8788x; 3.8788x over previous
"""Pairwise IoU (8192x8192) on 8 Trainium2 NeuronCores via Bass/Tile.

Strategy
--------
boxes1 rows are split across 8 cores (1024 sorted rows each).  Both box
sets are sorted by x1 on the host; boxes with x-ranges further apart
than the max box width cannot overlap, so each 128-row i-tile only has
to be scored against a contiguous window of ~2.4k x1-sorted boxes2
columns (~29% of the 8192).  The rest of the output is exactly zero and
is assembled on the host.

The device program is SPMD (one program, 8 cores), so the per-i-tile
window offsets are fixed compile-time constants off_t = t*S; the host
packs each core's column window so that tile t's true window lies
inside [base_c + t*S, base_c + t*S + W).  S and W are derived from the
actual data at first call.

Per-core device kernel, per [128, W] output tile (4 DVE passes, no
TensorE/PSUM):
  rx    = relu(min(x2_i, X2_j) - max(x1_i, X1_j))    custom DVE op (fp16 out)
  ry    = same for y                                 custom DVE op (fp16 out)
  inter = rx*ry                                      DVE tensor_tensor (fp16, 2x)
  out   = inter * recip(a1_i + a2e_j - inter)        custom DVE op IOU_DIV:
          union + bitcast-NOT reciprocal seed + 1 Newton step + multiply
          fused into one 8-stage DVE pass (bf16 out, ~0.2% rel err)
"""

import numpy as np

N = 8192
M = 8192
NCORES = 8
ROWS = N // NCORES  # rows of boxes1 per core
P = 128  # partitions
NT = ROWS // P  # 8 i-tiles per core
EPS = 1e-7

# 1-Newton reciprocal constants (minimax over t = u*bitcast(~u) in [-4.5,-4])
RC0 = -0.23549792
RC1 = 2.0017324

_COMPILED = {}


def _register_op(name, spec, subdim=False):
    import concourse.dve_ops as dve_ops
    from concourse.dve_spec import lower
    from concourse.dve_uop import DveOpSpec

    for op in dve_ops.OPS:
        if op.name == name:
            return op
    shas = {}
    for ver in ("v3", "v4"):
        try:
            shas[ver] = DveOpSpec(
                name=name, opcode=0, uops=lower(spec, ver=ver)
            ).sha(ver)
        except Exception:
            pass
    op = dve_ops.DveOp(name, spec, subdim=subdim, uops_sha=shas)
    dve_ops.OPS.append(op)
    dve_ops.CUSTOM_DVE_SPECS[op.name] = op.spec
    dve_ops._SUB_OPCODE_FOR_NAME[op.name] = (
        dve_ops._CUSTOM_DVE_ROW_BASE + len(dve_ops.OPS) - 1
    )
    return op


def _np_recip1(u):
    nu = (~np.asarray(u, np.float32).view(np.int32)).view(np.float32)
    y0 = (nu * np.float32(RC0)).astype(np.float32)
    return (y0 * (np.float32(RC1) - u * y0)).astype(np.float32)


def _ensure_ops():
    """Register the IOU_EDGE and IOU_DIV custom DVE ops (idempotent)."""
    from concourse.dve_spec import (
        C0,
        C1,
        C2,
        AluOp,
        Bin,
        Spec,
        Src0,
        Src1,
        maxx,
        minn,
        relu,
    )

    edge = _register_op(
        "IOU_EDGE",
        Spec(
            body=relu(minn(Src1, C1) - maxx(Src0, C0)),
            reference=lambda in0, in1, s0, s1, imm2: np.maximum(
                np.minimum(in1, s1) - np.maximum(in0, s0), 0.0
            ).astype(np.float32),
        ),
    )

    # u = (a1 + a2e) - inter; out = inter * recip1NR(u).  8 ALU stages.
    _t1 = C0 + Src1
    _u = _t1 - Src0
    _nu = Bin(AluOp.BITWISE_NOT, _u, _u)
    _y0 = _nu * C1
    _y1 = _y0 * (C2 - _u * _y0)
    div = _register_op(
        "IOU_DIV",
        Spec(
            body=Src0 * _y1,
            reference=lambda in0, in1, s0, s1, imm2: (
                in0 * _np_recip1((s0 + in1) - in0)
            ).astype(np.float32),
        ),
    )
    return edge, div


def _build_program(W, S, WCOL):
    from contextlib import ExitStack

    import concourse.bacc as bacc
    import concourse.mybir as mybir
    import concourse.tile as tile

    iou_edge, iou_div = _ensure_ops()

    f32 = mybir.dt.float32
    f16 = mybir.dt.float16
    bf16 = mybir.dt.bfloat16
    nc = bacc.Bacc(
        "TRN2",
        target_bir_lowering=False,
        debug=False,
        enable_asserts=False,
        num_devices=NCORES,
    )

    # DRAM I/O. boxes2 coord rows are host-replicated across partitions.
    x1b = nc.dram_tensor("x1b", [P, WCOL], f32, kind="ExternalInput").ap()
    x2b = nc.dram_tensor("x2b", [P, WCOL], f32, kind="ExternalInput").ap()
    y1b = nc.dram_tensor("y1b", [P, WCOL], f32, kind="ExternalInput").ap()
    y2b = nc.dram_tensor("y2b", [P, WCOL], f32, kind="ExternalInput").ap()
    a2eb = nc.dram_tensor("a2eb", [P, WCOL], f16, kind="ExternalInput").ap()
    # Per-partition scalars: for i-tile t, columns t*5+k hold
    # (x1, x2, y1, y2, area1) of sorted boxes1 row t*128+p.
    sc = nc.dram_tensor("sc", [P, NT * 5], f32, kind="ExternalInput").ap()
    out = nc.dram_tensor("out", [ROWS, W], bf16, kind="ExternalOutput").ap()

    with tile.TileContext(nc) as tc, ExitStack() as ctx:
        bc = ctx.enter_context(tc.tile_pool(name="bc", bufs=1))
        scp = ctx.enter_context(tc.tile_pool(name="scp", bufs=1))
        work = ctx.enter_context(tc.tile_pool(name="work", bufs=2))
        outp = ctx.enter_context(tc.tile_pool(name="outp", bufs=3))

        sct = scp.tile([P, NT * 5], f32)
        nc.sync.dma_start(sct[:], sc[:])
        x1t = bc.tile([P, WCOL], f32)
        x2t = bc.tile([P, WCOL], f32)
        y1t = bc.tile([P, WCOL], f32)
        y2t = bc.tile([P, WCOL], f32)
        a2et = bc.tile([P, WCOL], f16)
        # chunked loads so early tiles' compute can start sooner
        H = WCOL // 2
        for tt, src in ((x1t, x1b), (x2t, x2b)):
            nc.sync.dma_start(tt[:, :H], src[:, :H])
        for tt, src in ((y1t, y1b), (y2t, y2b), (a2et, a2eb)):
            nc.sync.dma_start(tt[:, :H], src[:, :H])
        for tt, src in (
            (x1t, x1b),
            (x2t, x2b),
            (y1t, y1b),
            (y2t, y2b),
            (a2et, a2eb),
        ):
            nc.sync.dma_start(tt[:, H:], src[:, H:])

        for t in range(NT):
            o = t * S
            c = t * 5
            rx = work.tile([P, W], f16, tag="rx")
            ry = work.tile([P, W], f16, tag="ry")
            inter = work.tile([P, W], f16, tag="inter")
            ot = outp.tile([P, W], bf16, tag="ot")

            nc.vector._custom_dve(
                iou_edge,
                out=rx[:],
                in0=x1t[:, o : o + W],
                in1=x2t[:, o : o + W],
                s0=sct[:, c : c + 1],
                s1=sct[:, c + 1 : c + 2],
            )
            nc.vector._custom_dve(
                iou_edge,
                out=ry[:],
                in0=y1t[:, o : o + W],
                in1=y2t[:, o : o + W],
                s0=sct[:, c + 2 : c + 3],
                s1=sct[:, c + 3 : c + 4],
            )
            nc.vector.tensor_mul(inter[:], rx[:], ry[:])
            nc.vector._custom_dve(
                iou_div,
                out=ot[:],
                in0=inter[:],
                in1=a2et[:, o : o + W],
                s0=sct[:, c + 4 : c + 5],
                s1=RC0,
                imm2=RC1,
            )
            nc.sync.dma_start(out[t * P : (t + 1) * P, :], ot[:])

    nc.compile()
    return nc


def _get_program(W, S, WCOL):
    key = (W, S, WCOL)
    if key not in _COMPILED:
        _COMPILED[key] = _build_program(W, S, WCOL)
    return _COMPILED[key]


def _plan(boxes1, boxes2):
    """Sort boxes, derive per-tile column windows and the (S, W) packing."""
    b1 = np.ascontiguousarray(boxes1, dtype=np.float32)
    b2 = np.ascontiguousarray(boxes2, dtype=np.float32)
    p1 = np.argsort(b1[:, 0], kind="stable")
    p2 = np.argsort(b2[:, 0], kind="stable")
    s1 = b1[p1]
    s2 = b2[p2]
    X1 = s2[:, 0]
    wmax2 = float((s2[:, 2] - s2[:, 0]).max())

    jL = np.empty((NCORES, NT), np.int64)
    jR = np.empty((NCORES, NT), np.int64)
    for c in range(NCORES):
        for t in range(NT):
            rows = s1[c * ROWS + t * P : c * ROWS + (t + 1) * P]
            lo = float(rows[:, 0].min())
            hi = float(rows[:, 2].max())
            jL[c, t] = np.searchsorted(X1, np.float32(lo - wmax2) - 1e-3)
            jR[c, t] = np.searchsorted(X1, np.float32(hi) + 1e-3)

    ts = np.arange(NT)
    best = None
    for S in range(0, 513, 16):
        l = jL - S * ts
        r = jR - S * ts
        Wneed = int((r.max(axis=1) - l.min(axis=1)).max())
        if best is None or Wneed < best[0]:
            best = (Wneed, S)
    Wneed, S = best
    W = min(-(-max(Wneed, 64) // 64) * 64, M + 512)
    WCOL = (NT - 1) * S + W
    bases = (jL - S * ts).min(axis=1)  # per-core packed-column origin
    return dict(
        b1=b1, b2=b2, p1=p1, p2=p2, s1=s1, s2=s2,
        W=W, S=S, WCOL=WCOL, bases=bases,
    )


def _make_in_maps(plan):
    s1, s2 = plan["s1"], plan["s2"]
    W, S, WCOL, bases = plan["W"], plan["S"], plan["WCOL"], plan["bases"]

    X1, Y1, X2, Y2 = s2[:, 0], s2[:, 1], s2[:, 2], s2[:, 3]
    a2e = ((X2 - X1) * (Y2 - Y1) + np.float32(EPS)).astype(np.float32)

    in_maps = []
    for c in range(NCORES):
        idx = bases[c] + np.arange(WCOL)
        valid = (idx >= 0) & (idx < M)
        idxc = np.clip(idx, 0, M - 1)
        pad = np.float32(-1e6)

        def rep(vec, fill, dt=np.float32):
            row = np.where(valid, vec[idxc], fill).astype(dt)
            return np.ascontiguousarray(np.broadcast_to(row, (P, WCOL)))

        m = {
            "x1b": rep(X1, pad),
            "x2b": rep(X2, pad),
            "y1b": rep(Y1, pad),
            "y2b": rep(Y2, pad),
            "a2eb": rep(a2e, np.float32(1.0), np.float16),
        }
        rows = s1[c * ROWS : (c + 1) * ROWS].reshape(NT, P, 4)
        a1 = (rows[:, :, 2] - rows[:, :, 0]) * (rows[:, :, 3] - rows[:, :, 1])
        scv = np.empty((P, NT * 5), dtype=np.float32)
        for t in range(NT):
            scv[:, t * 5 + 0] = rows[t, :, 0]
            scv[:, t * 5 + 1] = rows[t, :, 2]
            scv[:, t * 5 + 2] = rows[t, :, 1]
            scv[:, t * 5 + 3] = rows[t, :, 3]
            scv[:, t * 5 + 4] = a1[t]
        m["sc"] = scv
        in_maps.append(m)
    return in_maps


def _assemble(plan, results):
    """Paste per-core [ROWS, W] bf16 blocks into the full fp32 matrix."""
    W, S, bases = plan["W"], plan["S"], plan["bases"]
    p1, p2 = plan["p1"], plan["p2"]

    out_sorted = np.zeros((N, M), dtype=np.float32)
    for c in range(NCORES):
        blk = results[c]["out"]  # [ROWS, W] bf16/fp32-convertible
        blk = np.asarray(blk)
        for t in range(NT):
            c0 = bases[c] + t * S
            c1 = c0 + W
            s0 = max(0, -c0)
            cc0 = max(0, c0)
            cc1 = min(M, c1)
            if cc1 <= cc0:
                continue
            out_sorted[
                c * ROWS + t * P : c * ROWS + (t + 1) * P, cc0:cc1
            ] = blk[t * P : (t + 1) * P, s0 : s0 + (cc1 - cc0)].astype(
                np.float32
            )

    inv1 = np.empty(N, np.int64)
    inv1[p1] = np.arange(N)
    inv2 = np.empty(M, np.int64)
    inv2[p2] = np.arange(M)
    tmp = out_sorted[inv1]
    return np.take(tmp, inv2, axis=1)


def _run(inputs, trace=False, tmpdir=None):
    from concourse.bass_utils import run_bass_kernel_spmd

    plan = _plan(inputs["boxes1"], inputs["boxes2"])
    nc = _get_program(plan["W"], plan["S"], plan["WCOL"])
    in_maps = _make_in_maps(plan)
    kwargs = {}
    if trace:
        kwargs = dict(trace=True, tmpdir=tmpdir)
    res = run_bass_kernel_spmd(
        nc, in_maps, core_ids=list(range(NCORES)), **kwargs
    )
    return plan, res


def kernel(boxes1: np.ndarray, boxes2: np.ndarray) -> np.ndarray:
    plan, res = _run({"boxes1": boxes1, "boxes2": boxes2})
    return _assemble(plan, res.results)


# revision 5
# speedup vs baseline: 4.5889x; 1.1831x over previous
"""Pairwise IoU (8192x8192) on 8 Trainium2 NeuronCores via Bass/Tile.

Strategy
--------
boxes1 rows are split across 8 cores (1024 sorted rows each).  Both box
sets are sorted by x1 on the host; boxes with x-ranges further apart
than the max box width cannot overlap, so each 128-row i-tile only has
to be scored against a contiguous window of ~2.4k x1-sorted boxes2
columns (~29% of the 8192).  The rest of the output is exactly zero and
is assembled on the host.

The device program is SPMD (one program, 8 cores), so the per-i-tile
window offsets are fixed compile-time constants OFFS[t]; the host packs
each core's column window so that tile t's true window lies inside
[base_c + OFFS[t], base_c + OFFS[t] + W).  OFFS and W are derived from
the actual data at first call.

Per-core device kernel, per [128, W] output tile:
  rx    = relu(min(x2_i, X2_j) - max(x1_i, X1_j))   custom DVE op (fp16 out)
  ry    = same for y                                custom DVE op (fp16 out)
  inter = rx*ry                                     DVE tensor_tensor (fp16, 2x)
  u     = a1_i + a2e_j - inter                      TensorE (fp16 rank-2 +
                                                    (-I)@inter) -> PSUM fp32
  rinv  = Exp(-Ln(u))                               ScalarE LUTs (fp16 out)
  out   = inter * rinv                              DVE tensor_tensor (bf16 out)
"""

import numpy as np

N = 8192
M = 8192
NCORES = 8
ROWS = N // NCORES  # rows of boxes1 per core
P = 128  # partitions
NT = ROWS // P  # 8 i-tiles per core
PS = 512  # psum bank width (fp32)
EPS = 1e-7

# 1-Newton reciprocal constants (fallback DVE div path)
RC0 = -0.23549792
RC1 = 2.0017324

USE_SCALAR_DIV = True  # TensorE union + ScalarE ln/exp + DVE mul
GPSIMD_INTER = False  # compute inter on GpSimd instead of DVE

_COMPILED = {}


def _register_op(name, spec, subdim=False):
    import concourse.dve_ops as dve_ops
    from concourse.dve_spec import lower
    from concourse.dve_uop import DveOpSpec

    for op in dve_ops.OPS:
        if op.name == name:
            return op
    shas = {}
    for ver in ("v3", "v4"):
        try:
            shas[ver] = DveOpSpec(
                name=name, opcode=0, uops=lower(spec, ver=ver)
            ).sha(ver)
        except Exception:
            pass
    op = dve_ops.DveOp(name, spec, subdim=subdim, uops_sha=shas)
    dve_ops.OPS.append(op)
    dve_ops.CUSTOM_DVE_SPECS[op.name] = op.spec
    dve_ops._SUB_OPCODE_FOR_NAME[op.name] = (
        dve_ops._CUSTOM_DVE_ROW_BASE + len(dve_ops.OPS) - 1
    )
    return op


def _np_recip1(u):
    nu = (~np.asarray(u, np.float32).view(np.int32)).view(np.float32)
    y0 = (nu * np.float32(RC0)).astype(np.float32)
    return (y0 * (np.float32(RC1) - u * y0)).astype(np.float32)


def _ensure_ops():
    """Register the IOU_EDGE and IOU_DIV custom DVE ops (idempotent)."""
    from concourse.dve_spec import (
        C0,
        C1,
        C2,
        AluOp,
        Bin,
        Spec,
        Src0,
        Src1,
        maxx,
        minn,
        relu,
    )

    edge = _register_op(
        "IOU_EDGE",
        Spec(
            body=relu(minn(Src1, C1) - maxx(Src0, C0)),
            reference=lambda in0, in1, s0, s1, imm2: np.maximum(
                np.minimum(in1, s1) - np.maximum(in0, s0), 0.0
            ).astype(np.float32),
        ),
    )

    # u = (a1 + a2e) - inter; out = inter * recip1NR(u).  8 ALU stages.
    _t1 = C0 + Src1
    _u = _t1 - Src0
    _nu = Bin(AluOp.BITWISE_NOT, _u, _u)
    _y0 = _nu * C1
    _y1 = _y0 * (C2 - _u * _y0)
    div = _register_op(
        "IOU_DIV",
        Spec(
            body=Src0 * _y1,
            reference=lambda in0, in1, s0, s1, imm2: (
                in0 * _np_recip1((s0 + in1) - in0)
            ).astype(np.float32),
        ),
    )
    return edge, div


def _build_program(W, OFFS, WCOL):
    from contextlib import ExitStack

    import concourse.bacc as bacc
    import concourse.mybir as mybir
    import concourse.tile as tile

    iou_edge, iou_div = _ensure_ops()

    f32 = mybir.dt.float32
    f16 = mybir.dt.float16
    bf16 = mybir.dt.bfloat16
    act = mybir.ActivationFunctionType
    nc = bacc.Bacc(
        "TRN2",
        target_bir_lowering=False,
        debug=False,
        enable_asserts=False,
        num_devices=NCORES,
    )

    # DRAM I/O. boxes2 coord rows are host-replicated across partitions.
    x1b = nc.dram_tensor("x1b", [P, WCOL], f32, kind="ExternalInput").ap()
    x2b = nc.dram_tensor("x2b", [P, WCOL], f32, kind="ExternalInput").ap()
    y1b = nc.dram_tensor("y1b", [P, WCOL], f32, kind="ExternalInput").ap()
    y2b = nc.dram_tensor("y2b", [P, WCOL], f32, kind="ExternalInput").ap()
    # Per-partition scalars: for i-tile t, columns t*5+k hold
    # (x1, x2, y1, y2, area1) of sorted boxes1 row t*128+p.
    sc = nc.dram_tensor("sc", [P, NT * 5], f32, kind="ExternalInput").ap()
    if USE_SCALAR_DIV:
        # moving operand for the union matmul: row0 = ones, row1 = a2+eps
        a2e2 = nc.dram_tensor("a2e2", [2, WCOL], f16, kind="ExternalInput").ap()
        # stationary: row0 = area1 (per sorted row), row1 = ones
        sta = nc.dram_tensor("sta", [2, ROWS], f16, kind="ExternalInput").ap()
        negi = nc.dram_tensor("negi", [P, P], f16, kind="ExternalInput").ap()
    else:
        a2eb = nc.dram_tensor("a2eb", [P, WCOL], f16, kind="ExternalInput").ap()
    out = nc.dram_tensor("out", [ROWS, W], bf16, kind="ExternalOutput").ap()

    NCH = -(-W // PS)  # psum chunks per tile

    with tile.TileContext(nc) as tc, ExitStack() as ctx:
        bc = ctx.enter_context(tc.tile_pool(name="bc", bufs=1))
        scp = ctx.enter_context(tc.tile_pool(name="scp", bufs=1))
        work = ctx.enter_context(tc.tile_pool(name="work", bufs=2))
        outp = ctx.enter_context(tc.tile_pool(name="outp", bufs=3))
        if USE_SCALAR_DIV:
            psum = ctx.enter_context(
                tc.tile_pool(name="psum", bufs=1, space="PSUM")
            )

        sct = scp.tile([P, NT * 5], f32)
        x1t = bc.tile([P, WCOL], f32)
        x2t = bc.tile([P, WCOL], f32)
        y1t = bc.tile([P, WCOL], f32)
        y2t = bc.tile([P, WCOL], f32)
        if USE_SCALAR_DIV:
            a2e2t = scp.tile([2, WCOL], f16)
            stat = scp.tile([2, ROWS], f16)
            negit = scp.tile([P, P], f16)
        else:
            a2et = bc.tile([P, WCOL], f16)

        # Load order: tile-0 windows of x (then y) coords first so compute
        # starts early; tails follow.  Spread the two head loads across the
        # two HWDGE queues (sync + scalar).
        nc.sync.dma_start(sct[:], sc[:])
        nc.scalar.dma_start(x2t[:, :W], x2b[:, :W])
        nc.sync.dma_start(x1t[:, :W], x1b[:, :W])
        if USE_SCALAR_DIV:
            nc.scalar.dma_start(stat[:], sta[:])
            nc.scalar.dma_start(negit[:], negi[:])
            nc.scalar.dma_start(a2e2t[:], a2e2[:])
        nc.scalar.dma_start(y2t[:, :W], y2b[:, :W])
        nc.sync.dma_start(y1t[:, :W], y1b[:, :W])
        if not USE_SCALAR_DIV:
            nc.scalar.dma_start(a2et[:], a2eb[:])
        if WCOL > W:
            nc.sync.dma_start(x1t[:, W:], x1b[:, W:])
            nc.scalar.dma_start(x2t[:, W:], x2b[:, W:])
            nc.sync.dma_start(y1t[:, W:], y1b[:, W:])
            nc.scalar.dma_start(y2t[:, W:], y2b[:, W:])

        for t in range(NT):
            o = OFFS[t]
            c = t * 5
            rx = work.tile([P, W], f16, tag="rx")
            ry = work.tile([P, W], f16, tag="ry")
            inter = work.tile([P, W], f16, tag="inter")
            ot = outp.tile([P, W], bf16, tag="ot")

            nc.vector._custom_dve(
                iou_edge,
                out=rx[:],
                in0=x1t[:, o : o + W],
                in1=x2t[:, o : o + W],
                s0=sct[:, c : c + 1],
                s1=sct[:, c + 1 : c + 2],
            )
            nc.vector._custom_dve(
                iou_edge,
                out=ry[:],
                in0=y1t[:, o : o + W],
                in1=y2t[:, o : o + W],
                s0=sct[:, c + 2 : c + 3],
                s1=sct[:, c + 3 : c + 4],
            )
            if GPSIMD_INTER:
                nc.gpsimd.tensor_mul(inter[:], rx[:], ry[:])
            else:
                nc.vector.tensor_mul(inter[:], rx[:], ry[:])

            if USE_SCALAR_DIV:
                ua = work.tile([P, W], f32, tag="ua")
                rinv = work.tile([P, W], f16, tag="rinv")
                pts = []
                for k in range(NCH):
                    c0 = k * PS
                    c1 = min(W, c0 + PS)
                    pt = psum.tile([P, PS], f32, tag="pt", bufs=8)
                    pt = pt[:, : c1 - c0]
                    pts.append((pt, c0, c1))
                    nc.tensor.matmul(
                        pt[:],
                        stat[:, t * P : (t + 1) * P],
                        a2e2t[:, o + c0 : o + c1],
                        start=True,
                        stop=False,
                    )
                for pt, c0, c1 in pts:
                    nc.tensor.matmul(
                        pt[:],
                        negit[:],
                        inter[:, c0:c1],
                        start=False,
                        stop=True,
                    )
                for pt, c0, c1 in pts:
                    nc.scalar.activation(ua[:, c0:c1], pt[:], act.Ln)
                nc.scalar.activation(rinv[:], ua[:], act.Exp, scale=-1.0)
                nc.vector.tensor_mul(ot[:], inter[:], rinv[:])
            else:
                nc.vector._custom_dve(
                    iou_div,
                    out=ot[:],
                    in0=inter[:],
                    in1=a2et[:, o : o + W],
                    s0=sct[:, c + 4 : c + 5],
                    s1=RC0,
                    imm2=RC1,
                )
            nc.sync.dma_start(out[t * P : (t + 1) * P, :], ot[:])

    nc.compile()
    return nc


def _get_program(W, OFFS, WCOL):
    key = (W, tuple(OFFS), WCOL)
    if key not in _COMPILED:
        _COMPILED[key] = _build_program(W, list(OFFS), WCOL)
    return _COMPILED[key]


def _plan(boxes1, boxes2):
    """Sort boxes, derive per-tile column windows and the OFFS/W packing."""
    b1 = np.ascontiguousarray(boxes1, dtype=np.float32)
    b2 = np.ascontiguousarray(boxes2, dtype=np.float32)
    p1 = np.argsort(b1[:, 0], kind="stable")
    p2 = np.argsort(b2[:, 0], kind="stable")
    s1 = b1[p1]
    s2 = b2[p2]
    X1 = s2[:, 0]
    wmax2 = float((s2[:, 2] - s2[:, 0]).max())

    jL = np.empty((NCORES, NT), np.int64)
    jR = np.empty((NCORES, NT), np.int64)
    for c in range(NCORES):
        for t in range(NT):
            rows = s1[c * ROWS + t * P : c * ROWS + (t + 1) * P]
            lo = float(rows[:, 0].min())
            hi = float(rows[:, 2].max())
            jL[c, t] = np.searchsorted(X1, np.float32(lo - wmax2) - 1e-3)
            jR[c, t] = np.searchsorted(X1, np.float32(hi) + 1e-3)

    def wneed(offs):
        l = jL - offs[None, :]
        r = jR - offs[None, :]
        return int((r.max(axis=1) - l.min(axis=1)).max())

    ts = np.arange(NT)
    best = None
    for S in range(0, 513, 16):
        Wn = wneed(S * ts)
        if best is None or Wn < best[0]:
            best = (Wn, S * ts)
    # refine: per-tile offsets at the cross-core median of jL (even-rounded)
    med = np.median(jL - jL[:, :1], axis=0)
    cand = 2 * np.round((med - med.min()) / 2).astype(np.int64)
    Wn = wneed(cand)
    if Wn < best[0]:
        best = (Wn, cand)
    Wneed, offs = best
    W = min(-(-max(Wneed, 64) // 32) * 32, M + 512)
    offs = offs - offs.min()
    WCOL = int(offs.max()) + W
    bases = (jL - offs[None, :]).min(axis=1)  # per-core packed origin
    return dict(
        b1=b1, b2=b2, p1=p1, p2=p2, s1=s1, s2=s2,
        W=W, OFFS=[int(o) for o in offs], WCOL=WCOL, bases=bases,
    )


def _make_in_maps(plan):
    s1, s2 = plan["s1"], plan["s2"]
    W, OFFS, WCOL, bases = plan["W"], plan["OFFS"], plan["WCOL"], plan["bases"]

    X1, Y1, X2, Y2 = s2[:, 0], s2[:, 1], s2[:, 2], s2[:, 3]
    a2e = ((X2 - X1) * (Y2 - Y1) + np.float32(EPS)).astype(np.float32)

    in_maps = []
    for c in range(NCORES):
        idx = bases[c] + np.arange(WCOL)
        valid = (idx >= 0) & (idx < M)
        idxc = np.clip(idx, 0, M - 1)
        pad = np.float32(-1e6)

        def rep(vec, fill, dt=np.float32):
            row = np.where(valid, vec[idxc], fill).astype(dt)
            return np.ascontiguousarray(np.broadcast_to(row, (P, WCOL)))

        m = {
            "x1b": rep(X1, pad),
            "x2b": rep(X2, pad),
            "y1b": rep(Y1, pad),
            "y2b": rep(Y2, pad),
        }
        rows = s1[c * ROWS : (c + 1) * ROWS].reshape(NT, P, 4)
        a1 = (rows[:, :, 2] - rows[:, :, 0]) * (rows[:, :, 3] - rows[:, :, 1])
        scv = np.empty((P, NT * 5), dtype=np.float32)
        for t in range(NT):
            scv[:, t * 5 + 0] = rows[t, :, 0]
            scv[:, t * 5 + 1] = rows[t, :, 2]
            scv[:, t * 5 + 2] = rows[t, :, 1]
            scv[:, t * 5 + 3] = rows[t, :, 3]
            scv[:, t * 5 + 4] = a1[t]
        m["sc"] = scv
        if USE_SCALAR_DIV:
            a2row = np.where(valid, a2e[idxc], np.float32(1.0))
            m["a2e2"] = np.ascontiguousarray(
                np.stack([np.ones(WCOL, np.float32), a2row]).astype(np.float16)
            )
            m["sta"] = np.ascontiguousarray(
                np.stack([a1.reshape(ROWS), np.ones(ROWS, np.float32)]).astype(
                    np.float16
                )
            )
            m["negi"] = (-np.eye(P)).astype(np.float16)
        else:
            m["a2eb"] = rep(a2e, np.float32(1.0), np.float16)
        in_maps.append(m)
    return in_maps


def _assemble(plan, results):
    """Paste per-core [ROWS, W] bf16 blocks into the full fp32 matrix."""
    W, OFFS, bases = plan["W"], plan["OFFS"], plan["bases"]
    p1, p2 = plan["p1"], plan["p2"]

    out_sorted = np.zeros((N, M), dtype=np.float32)
    for c in range(NCORES):
        blk = np.asarray(results[c]["out"])
        for t in range(NT):
            c0 = bases[c] + OFFS[t]
            c1 = c0 + W
            s0 = max(0, -c0)
            cc0 = max(0, c0)
            cc1 = min(M, c1)
            if cc1 <= cc0:
                continue
            out_sorted[
                c * ROWS + t * P : c * ROWS + (t + 1) * P, cc0:cc1
            ] = blk[t * P : (t + 1) * P, s0 : s0 + (cc1 - cc0)].astype(
                np.float32
            )

    inv1 = np.empty(N, np.int64)
    inv1[p1] = np.arange(N)
    inv2 = np.empty(M, np.int64)
    inv2[p2] = np.arange(M)
    tmp = out_sorted[inv1]
    return np.take(tmp, inv2, axis=1)


def _run(inputs, trace=False, tmpdir=None):
    from concourse.bass_utils import run_bass_kernel_spmd

    plan = _plan(inputs["boxes1"], inputs["boxes2"])
    nc = _get_program(plan["W"], plan["OFFS"], plan["WCOL"])
    in_maps = _make_in_maps(plan)
    kwargs = {}
    if trace:
        kwargs = dict(trace=True, tmpdir=tmpdir)
    res = run_bass_kernel_spmd(
        nc, in_maps, core_ids=list(range(NCORES)), **kwargs
    )
    return plan, res


def kernel(boxes1: np.ndarray, boxes2: np.ndarray) -> np.ndarray:
    plan, res = _run({"boxes1": boxes1, "boxes2": boxes2})
    return _assemble(plan, res.results)


# revision 7
# speedup vs baseline: 5.0076x; 1.0912x over previous
"""Pairwise IoU (8192x8192) on 8 Trainium2 NeuronCores via Bass/Tile.

Strategy
--------
boxes1 rows are split across 8 cores (1024 sorted rows each).  Both box
sets are sorted by x1 on the host; boxes with x-ranges further apart
than the max box width cannot overlap, so each 128-row i-tile only has
to be scored against a contiguous window of ~2.4k x1-sorted boxes2
columns (~29% of the 8192).  The rest of the output is exactly zero and
is assembled on the host.

The device program is SPMD (one program, 8 cores), so the per-i-tile
window offsets are fixed compile-time constants OFFS[t]; the host packs
each core's column window so that tile t's true window lies inside
[base_c + OFFS[t], base_c + OFFS[t] + W).  OFFS and W are derived from
the actual data at first call.

Per-core device kernel, per [128, W] output tile:
  rx    = relu(min(x2_i, X2_j) - max(x1_i, X1_j))   custom DVE op (fp16 out)
  ry    = same for y                                custom DVE op (fp16 out)
  inter = rx*ry                                     DVE tensor_tensor (fp16, 2x)
  u     = a1_i + a2e_j - inter                      TensorE (fp16 rank-2 +
                                                    (-I)@inter) -> PSUM fp32
  rinv  = Exp(-Ln(u))                               ScalarE LUTs (fp16 out)
  out   = inter * rinv                              DVE tensor_tensor (bf16 out)
"""

import numpy as np

N = 8192
M = 8192
NCORES = 8
ROWS = N // NCORES  # rows of boxes1 per core
P = 128  # partitions
NT = ROWS // P  # 8 i-tiles per core
PS = 512  # psum bank width (fp32)
EPS = 1e-7

# 1-Newton reciprocal constants (fallback DVE div path)
RC0 = -0.23549792
RC1 = 2.0017324

USE_SCALAR_DIV = True  # TensorE union + ScalarE ln/exp + DVE mul
GPSIMD_INTER = False  # compute inter on GpSimd instead of DVE

_COMPILED = {}


def _register_op(name, spec, subdim=False):
    import concourse.dve_ops as dve_ops
    from concourse.dve_spec import lower
    from concourse.dve_uop import DveOpSpec

    for op in dve_ops.OPS:
        if op.name == name:
            return op
    shas = {}
    for ver in ("v3", "v4"):
        try:
            shas[ver] = DveOpSpec(
                name=name, opcode=0, uops=lower(spec, ver=ver)
            ).sha(ver)
        except Exception:
            pass
    op = dve_ops.DveOp(name, spec, subdim=subdim, uops_sha=shas)
    dve_ops.OPS.append(op)
    dve_ops.CUSTOM_DVE_SPECS[op.name] = op.spec
    dve_ops._SUB_OPCODE_FOR_NAME[op.name] = (
        dve_ops._CUSTOM_DVE_ROW_BASE + len(dve_ops.OPS) - 1
    )
    return op


def _np_recip1(u):
    nu = (~np.asarray(u, np.float32).view(np.int32)).view(np.float32)
    y0 = (nu * np.float32(RC0)).astype(np.float32)
    return (y0 * (np.float32(RC1) - u * y0)).astype(np.float32)


def _ensure_ops():
    """Register the IOU_EDGE and IOU_DIV custom DVE ops (idempotent)."""
    from concourse.dve_spec import (
        C0,
        C1,
        C2,
        AluOp,
        Bin,
        Spec,
        Src0,
        Src1,
        maxx,
        minn,
        relu,
    )

    edge = _register_op(
        "IOU_EDGE",
        Spec(
            body=relu(minn(Src1, C1) - maxx(Src0, C0)),
            reference=lambda in0, in1, s0, s1, imm2: np.maximum(
                np.minimum(in1, s1) - np.maximum(in0, s0), 0.0
            ).astype(np.float32),
        ),
    )

    # u = (a1 + a2e) - inter; out = inter * recip1NR(u).  8 ALU stages.
    _t1 = C0 + Src1
    _u = _t1 - Src0
    _nu = Bin(AluOp.BITWISE_NOT, _u, _u)
    _y0 = _nu * C1
    _y1 = _y0 * (C2 - _u * _y0)
    div = _register_op(
        "IOU_DIV",
        Spec(
            body=Src0 * _y1,
            reference=lambda in0, in1, s0, s1, imm2: (
                in0 * _np_recip1((s0 + in1) - in0)
            ).astype(np.float32),
        ),
    )
    return edge, div


def _build_program(W, OFFS, WCOL):
    from contextlib import ExitStack

    import concourse.bacc as bacc
    import concourse.mybir as mybir
    import concourse.tile as tile

    iou_edge, iou_div = _ensure_ops()

    f32 = mybir.dt.float32
    f16 = mybir.dt.float16
    bf16 = mybir.dt.bfloat16
    act = mybir.ActivationFunctionType
    nc = bacc.Bacc(
        "TRN2",
        target_bir_lowering=False,
        debug=False,
        enable_asserts=False,
        num_devices=NCORES,
    )

    if USE_SCALAR_DIV:
        # The default act-table placement resolves Ln and Exp to different
        # table sets, reloading tables on every switch (~2.7us each).  Route
        # both to the one set that contains them, preserving set indices.
        import types

        import bass_rust as _bass_rust
        from concourse.hw_specs import get_activation_tables

        def _insert_act_table_loads(self):
            has_activation = any(
                isinstance(i, mybir.InstActivation)
                for b in self.main_func.blocks
                for i in b.instructions
            )
            if not has_activation:
                return
            both = {act.Ln, act.Exp}
            tables = [
                (name, fns if both <= fns else fns - both)
                for name, fns in get_activation_tables(self.m.arch).items()
            ]
            _bass_rust.insert_act_table_loads(self, tables)

        nc.insert_act_table_loads = types.MethodType(_insert_act_table_loads, nc)

    # DRAM I/O. boxes2 coord rows are host-replicated across partitions.
    x1b = nc.dram_tensor("x1b", [P, WCOL], f32, kind="ExternalInput").ap()
    x2b = nc.dram_tensor("x2b", [P, WCOL], f32, kind="ExternalInput").ap()
    y1b = nc.dram_tensor("y1b", [P, WCOL], f32, kind="ExternalInput").ap()
    y2b = nc.dram_tensor("y2b", [P, WCOL], f32, kind="ExternalInput").ap()
    # Per-partition scalars: for i-tile t, columns t*5+k hold
    # (x1, x2, y1, y2, area1) of sorted boxes1 row t*128+p.
    sc = nc.dram_tensor("sc", [P, NT * 5], f32, kind="ExternalInput").ap()
    if USE_SCALAR_DIV:
        # moving operand for the union matmul: row0 = ones, row1 = a2+eps
        a2e2 = nc.dram_tensor("a2e2", [2, WCOL], f16, kind="ExternalInput").ap()
        # stationary: row0 = area1 (per sorted row), row1 = ones
        sta = nc.dram_tensor("sta", [2, ROWS], f16, kind="ExternalInput").ap()
        negi = nc.dram_tensor("negi", [P, P], f16, kind="ExternalInput").ap()
    else:
        a2eb = nc.dram_tensor("a2eb", [P, WCOL], f16, kind="ExternalInput").ap()
    out = nc.dram_tensor("out", [ROWS, W], bf16, kind="ExternalOutput").ap()

    NCH = -(-W // PS)  # psum chunks per tile

    with tile.TileContext(nc) as tc, ExitStack() as ctx:
        bc = ctx.enter_context(tc.tile_pool(name="bc", bufs=1))
        scp = ctx.enter_context(tc.tile_pool(name="scp", bufs=1))
        work = ctx.enter_context(tc.tile_pool(name="work", bufs=2))
        outp = ctx.enter_context(tc.tile_pool(name="outp", bufs=3))
        if USE_SCALAR_DIV:
            psum = ctx.enter_context(
                tc.tile_pool(name="psum", bufs=1, space="PSUM")
            )

        sct = scp.tile([P, NT * 5], f32)
        x1t = bc.tile([P, WCOL], f32)
        x2t = bc.tile([P, WCOL], f32)
        y1t = bc.tile([P, WCOL], f32)
        y2t = bc.tile([P, WCOL], f32)
        if USE_SCALAR_DIV:
            a2e2t = scp.tile([2, WCOL], f16)
            stat = scp.tile([2, ROWS], f16)
            negit = scp.tile([P, P], f16)
        else:
            a2et = bc.tile([P, WCOL], f16)

        # Load order: tile-0 windows of x (then y) coords first so compute
        # starts early; tails follow.  Spread the two head loads across the
        # two HWDGE queues (sync + scalar).
        nc.sync.dma_start(sct[:], sc[:])
        nc.scalar.dma_start(x2t[:, :W], x2b[:, :W])
        nc.sync.dma_start(x1t[:, :W], x1b[:, :W])
        nc.scalar.dma_start(y2t[:, :W], y2b[:, :W])
        nc.sync.dma_start(y1t[:, :W], y1b[:, :W])
        if USE_SCALAR_DIV:
            nc.scalar.dma_start(stat[:], sta[:])
            nc.scalar.dma_start(negit[:], negi[:])
            nc.scalar.dma_start(a2e2t[:], a2e2[:])
        else:
            nc.scalar.dma_start(a2et[:], a2eb[:])
        if WCOL > W:
            nc.sync.dma_start(x1t[:, W:], x1b[:, W:])
            nc.scalar.dma_start(x2t[:, W:], x2b[:, W:])
            nc.sync.dma_start(y1t[:, W:], y1b[:, W:])
            nc.scalar.dma_start(y2t[:, W:], y2b[:, W:])

        for t in range(NT):
            o = OFFS[t]
            c = t * 5
            rx = work.tile([P, W], f16, tag="rx")
            ry = work.tile([P, W], f16, tag="ry")
            inter = work.tile([P, W], f16, tag="inter")
            ot = outp.tile([P, W], bf16, tag="ot")

            nc.vector._custom_dve(
                iou_edge,
                out=rx[:],
                in0=x1t[:, o : o + W],
                in1=x2t[:, o : o + W],
                s0=sct[:, c : c + 1],
                s1=sct[:, c + 1 : c + 2],
            )
            nc.vector._custom_dve(
                iou_edge,
                out=ry[:],
                in0=y1t[:, o : o + W],
                in1=y2t[:, o : o + W],
                s0=sct[:, c + 2 : c + 3],
                s1=sct[:, c + 3 : c + 4],
            )
            if GPSIMD_INTER:
                nc.gpsimd.tensor_mul(inter[:], rx[:], ry[:])
            else:
                nc.vector.tensor_mul(inter[:], rx[:], ry[:])

            if USE_SCALAR_DIV:
                ua = work.tile([P, W], f32, tag="ua")
                rinv = work.tile([P, W], f16, tag="rinv")
                pts = []
                for k in range(NCH):
                    c0 = k * PS
                    c1 = min(W, c0 + PS)
                    pt = psum.tile([P, PS], f32, tag="pt", bufs=8)
                    pt = pt[:, : c1 - c0]
                    pts.append((pt, c0, c1))
                    nc.tensor.matmul(
                        pt[:],
                        stat[:, t * P : (t + 1) * P],
                        a2e2t[:, o + c0 : o + c1],
                        start=True,
                        stop=False,
                    )
                for pt, c0, c1 in pts:
                    nc.tensor.matmul(
                        pt[:],
                        negit[:],
                        inter[:, c0:c1],
                        start=False,
                        stop=True,
                    )
                for pt, c0, c1 in pts:
                    nc.scalar.activation(ua[:, c0:c1], pt[:], act.Ln)
                nc.scalar.activation(rinv[:], ua[:], act.Exp, scale=-1.0)
                nc.vector.tensor_mul(ot[:], inter[:], rinv[:])
            else:
                nc.vector._custom_dve(
                    iou_div,
                    out=ot[:],
                    in0=inter[:],
                    in1=a2et[:, o : o + W],
                    s0=sct[:, c + 4 : c + 5],
                    s1=RC0,
                    imm2=RC1,
                )
            nc.sync.dma_start(out[t * P : (t + 1) * P, :], ot[:])

    nc.compile()
    return nc


def _get_program(W, OFFS, WCOL):
    key = (W, tuple(OFFS), WCOL)
    if key not in _COMPILED:
        _COMPILED[key] = _build_program(W, list(OFFS), WCOL)
    return _COMPILED[key]


def _plan(boxes1, boxes2):
    """Sort boxes, derive per-tile column windows and the OFFS/W packing."""
    b1 = np.ascontiguousarray(boxes1, dtype=np.float32)
    b2 = np.ascontiguousarray(boxes2, dtype=np.float32)
    p1 = np.argsort(b1[:, 0], kind="stable")
    p2 = np.argsort(b2[:, 0], kind="stable")
    s1 = b1[p1]
    s2 = b2[p2]
    X1 = s2[:, 0]
    wmax2 = float((s2[:, 2] - s2[:, 0]).max())

    jL = np.empty((NCORES, NT), np.int64)
    jR = np.empty((NCORES, NT), np.int64)
    for c in range(NCORES):
        for t in range(NT):
            rows = s1[c * ROWS + t * P : c * ROWS + (t + 1) * P]
            lo = float(rows[:, 0].min())
            hi = float(rows[:, 2].max())
            jL[c, t] = np.searchsorted(X1, np.float32(lo - wmax2) - 1e-3)
            jR[c, t] = np.searchsorted(X1, np.float32(hi) + 1e-3)

    def wneed(offs):
        l = jL - offs[None, :]
        r = jR - offs[None, :]
        return int((r.max(axis=1) - l.min(axis=1)).max())

    ts = np.arange(NT)
    best = None
    for S in range(0, 513, 16):
        Wn = wneed(S * ts)
        if best is None or Wn < best[0]:
            best = (Wn, S * ts)
    # refine: per-tile offsets at the cross-core median of jL (even-rounded)
    med = np.median(jL - jL[:, :1], axis=0)
    cand = 2 * np.round((med - med.min()) / 2).astype(np.int64)
    Wn = wneed(cand)
    if Wn < best[0]:
        best = (Wn, cand)
    Wneed, offs = best
    W = min(-(-max(Wneed, 64) // 32) * 32, M + 512)
    offs = offs - offs.min()
    WCOL = int(offs.max()) + W
    bases = (jL - offs[None, :]).min(axis=1)  # per-core packed origin
    return dict(
        b1=b1, b2=b2, p1=p1, p2=p2, s1=s1, s2=s2,
        W=W, OFFS=[int(o) for o in offs], WCOL=WCOL, bases=bases,
    )


def _make_in_maps(plan):
    s1, s2 = plan["s1"], plan["s2"]
    W, OFFS, WCOL, bases = plan["W"], plan["OFFS"], plan["WCOL"], plan["bases"]

    X1, Y1, X2, Y2 = s2[:, 0], s2[:, 1], s2[:, 2], s2[:, 3]
    a2e = ((X2 - X1) * (Y2 - Y1) + np.float32(EPS)).astype(np.float32)

    in_maps = []
    for c in range(NCORES):
        idx = bases[c] + np.arange(WCOL)
        valid = (idx >= 0) & (idx < M)
        idxc = np.clip(idx, 0, M - 1)
        pad = np.float32(-1e6)

        def rep(vec, fill, dt=np.float32):
            row = np.where(valid, vec[idxc], fill).astype(dt)
            return np.ascontiguousarray(np.broadcast_to(row, (P, WCOL)))

        m = {
            "x1b": rep(X1, pad),
            "x2b": rep(X2, pad),
            "y1b": rep(Y1, pad),
            "y2b": rep(Y2, pad),
        }
        rows = s1[c * ROWS : (c + 1) * ROWS].reshape(NT, P, 4)
        a1 = (rows[:, :, 2] - rows[:, :, 0]) * (rows[:, :, 3] - rows[:, :, 1])
        scv = np.empty((P, NT * 5), dtype=np.float32)
        for t in range(NT):
            scv[:, t * 5 + 0] = rows[t, :, 0]
            scv[:, t * 5 + 1] = rows[t, :, 2]
            scv[:, t * 5 + 2] = rows[t, :, 1]
            scv[:, t * 5 + 3] = rows[t, :, 3]
            scv[:, t * 5 + 4] = a1[t]
        m["sc"] = scv
        if USE_SCALAR_DIV:
            a2row = np.where(valid, a2e[idxc], np.float32(1.0))
            m["a2e2"] = np.ascontiguousarray(
                np.stack([np.ones(WCOL, np.float32), a2row]).astype(np.float16)
            )
            m["sta"] = np.ascontiguousarray(
                np.stack([a1.reshape(ROWS), np.ones(ROWS, np.float32)]).astype(
                    np.float16
                )
            )
            m["negi"] = (-np.eye(P)).astype(np.float16)
        else:
            m["a2eb"] = rep(a2e, np.float32(1.0), np.float16)
        in_maps.append(m)
    return in_maps


def _assemble(plan, results):
    """Paste per-core [ROWS, W] bf16 blocks into the full fp32 matrix."""
    W, OFFS, bases = plan["W"], plan["OFFS"], plan["bases"]
    p1, p2 = plan["p1"], plan["p2"]

    out_sorted = np.zeros((N, M), dtype=np.float32)
    for c in range(NCORES):
        blk = np.asarray(results[c]["out"])
        for t in range(NT):
            c0 = bases[c] + OFFS[t]
            c1 = c0 + W
            s0 = max(0, -c0)
            cc0 = max(0, c0)
            cc1 = min(M, c1)
            if cc1 <= cc0:
                continue
            out_sorted[
                c * ROWS + t * P : c * ROWS + (t + 1) * P, cc0:cc1
            ] = blk[t * P : (t + 1) * P, s0 : s0 + (cc1 - cc0)].astype(
                np.float32
            )

    inv1 = np.empty(N, np.int64)
    inv1[p1] = np.arange(N)
    inv2 = np.empty(M, np.int64)
    inv2[p2] = np.arange(M)
    tmp = out_sorted[inv1]
    return np.take(tmp, inv2, axis=1)


def _run(inputs, trace=False, tmpdir=None):
    from concourse.bass_utils import run_bass_kernel_spmd

    plan = _plan(inputs["boxes1"], inputs["boxes2"])
    nc = _get_program(plan["W"], plan["OFFS"], plan["WCOL"])
    in_maps = _make_in_maps(plan)
    kwargs = {}
    if trace:
        kwargs = dict(trace=True, tmpdir=tmpdir)
    res = run_bass_kernel_spmd(
        nc, in_maps, core_ids=list(range(NCORES)), **kwargs
    )
    return plan, res


def kernel(boxes1: np.ndarray, boxes2: np.ndarray) -> np.ndarray:
    plan, res = _run({"boxes1": boxes1, "boxes2": boxes2})
    return _assemble(plan, res.results)
